# revision 8
# baseline (speedup 1.0000x reference)
"""DigitCaps dynamic-routing kernel for Trainium2, 8 NeuronCores (SPMD).

Problem:  in_caps [64, 2048, 16] f32, W [2048, 32, 32, 16] f32
          u_hat[b,r,j,o] = sum_i W[r,j,o,i] * in_caps[b,r,i]
          3 routing iterations:
            c = softmax_j(b_ij)                     # b_ij [R, J]
            s[b,j,o] = sum_r c[r,j] u_hat[b,r,j,o]
            v = squash_o(s)
            b_ij += (1/BS) sum_{b,o} u_hat[b,r,j,o] v[b,j,o]
          returns v[..., None]  -> [64, 32, 32, 1]

Strategy (per core, routes sharded 256/core; K = (r,i) = 4096 rows):
  * W shard stays SBUF-resident as Wt[(r,i), (j,o)] = [4096, 1024] (128KB/part).
  * u_hat is never materialized.  Each iteration:
      pass 1: s = (c-scaled Wt).T-contracted with uT -> one K=4096 matmul
              streamed in 32 chunks of K=128 (moving operand = scaled W chunk,
              stationary = uT chunk).  f32r -> full PE rate.
              -> AllReduce of partial s [64, 1024] (the only collective).
      pass 2: G[(r,i),(j,o)] = sum_b u[b,(r,i)] v[b,(j,o)]   (PE, K=64)
              b_upd[r,j] = (1/64) sum_{i,o} Wt . G           (DVE mult +
              o-reduce + i-reduce via a constant block-selector matmul,
              accumulated directly into a persistent PSUM b tensor).
  * c_ij softmax is computed replicated over the 16 i-rows of each route so
    the scale of W chunks is a plain broadcast tensor_tensor multiply.
"""

import numpy as np

import concourse.bacc as bacc
import concourse.mybir as mybir
import concourse.tile as tile
from concourse.bass_utils import run_bass_kernel_spmd

BS, R, J, I, O = 64, 2048, 32, 16, 32
NUM_IT = 3
N_CORES = 8
R_LOC = R // N_CORES            # 256 routes per core
K_LOC = R_LOC * I               # 4096 contraction rows per core
NCHUNK = K_LOC // 128           # 32 chunks of 128 rows (8 routes x 16 i)
JO = J * O                      # 1024
F32 = mybir.dt.float32
F32R = mybir.dt.float32r
AX = mybir.AxisListType
ALU = mybir.AluOpType
ACTF = mybir.ActivationFunctionType

# o-reduction in pass 2: "mm_dup" folds it into the selector matmul via a
# duplicated (step-0) PSUM out AP; "dve" uses an explicit tensor_reduce.
O_REDUCE = "dve"
# chunks whose c-scale multiply runs on gpsimd instead of vector (pass 1)
WC_ON_GPSIMD = lambda c: c % 3 != 0
# chunks whose W.G multiply runs on gpsimd (needs an ACT psum->sbuf copy)
MULT_ON_GPSIMD = lambda c: c % 3 == 1


def _build_nc():
    nc = bacc.Bacc(trn_type="TRN2", target_bir_lowering=False, debug=False,
                   num_devices=N_CORES)
    wt = nc.dram_tensor("wt", [K_LOC, JO], F32R, kind="ExternalInput")
    ut = nc.dram_tensor("ut", [K_LOC, BS], F32R, kind="ExternalInput")
    un = nc.dram_tensor("un", [BS, K_LOC], F32R, kind="ExternalInput")
    sel = nc.dram_tensor("sel", [128, 128], F32, kind="ExternalInput")
    vout = nc.dram_tensor("vout", [BS, JO], F32, kind="ExternalOutput")
    cc_in = [nc.dram_tensor(f"cc_in{i}", [BS, JO], F32) for i in range(NUM_IT)]
    cc_out = [nc.dram_tensor(f"cc_out{i}", [BS, JO], F32, addr_space="Shared")
              for i in range(NUM_IT)]
    rg = [list(range(N_CORES))]

    with tile.TileContext(nc) as tc:
        with (
            tc.tile_pool(name="big", bufs=1) as big,
            tc.tile_pool(name="wc", bufs=3) as wcp,
            tc.tile_pool(name="tmp", bufs=2) as tmpp,
            tc.tile_pool(name="gsb", bufs=2) as gsbp,
            tc.tile_pool(name="small", bufs=1) as small,
            tc.tile_pool(name="spsum", bufs=1, space="PSUM") as spsum,
            tc.tile_pool(name="gpsum", bufs=2, space="PSUM") as gpsum,
            tc.tile_pool(name="bpsum", bufs=1, space="PSUM") as bpsum,
        ):
            # ---- resident tensors ----
            w_sb = big.tile([128, NCHUNK, JO], F32R)       # 128KB/part
            ut_sb = big.tile([128, NCHUNK, BS], F32R)      # 8KB/part
            un_sb = big.tile([BS, K_LOC], F32R)            # 16KB/part
            sel_sb = big.tile([128, 128], F32)            # selector (1/64)
            e_rep = big.tile([128, NCHUNK, J], F32)       # softmax scratch
            c_rep = big.tile([128, NCHUNK, J], F32)       # c_ij replicated
            b_acc = bpsum.tile([128, NCHUNK, J], F32)     # persistent b_ij

            wt_v = wt.ap().rearrange("(c p) f -> c p f", p=128)
            ut_v = ut.ap().rearrange("(c p) f -> c p f", p=128)
            for c in range(NCHUNK):
                nc.sync.dma_start(out=w_sb[:, c, :], in_=wt_v[c])
            for c in range(NCHUNK):
                nc.sync.dma_start(out=ut_sb[:, c, :], in_=ut_v[c])
            nc.sync.dma_start(out=un_sb, in_=un.ap())
            nc.sync.dma_start(out=sel_sb, in_=sel.ap())

            v_sb = None
            for it in range(NUM_IT):
                # ---------- pass 1: s = sum_{(r,i)} cW . uT ----------
                s_ps = spsum.tile([BS, JO], F32)
                for c in range(NCHUNK):
                    if it == 0:
                        rhs_src = w_sb[:, c, :]
                    else:
                        wc_t = wcp.tile([128, JO], F32R)
                        eng = nc.gpsimd if WC_ON_GPSIMD(c) else nc.vector
                        eng.tensor_tensor(
                            out=wc_t.rearrange("p (j o) -> p j o", o=O),
                            in0=w_sb[:, c, :].bitcast(F32).rearrange("p (j o) -> p j o", o=O),
                            in1=c_rep[:, c, :].unsqueeze(2).broadcast_to(
                                [128, J, O]),
                            op=ALU.mult,
                        )
                        rhs_src = wc_t
                    for h in range(2):
                        nc.tensor.matmul(
                            out=s_ps[:, h * 512:(h + 1) * 512],
                            lhsT=ut_sb[:, c, :],
                            rhs=rhs_src[:, h * 512:(h + 1) * 512],
                            start=(c == 0), stop=(c == NCHUNK - 1),
                        )
                # psum -> sbuf (iter 0 also applies the uniform c = 1/J)
                s_sb = small.tile([BS, JO], F32)
                if it == 0:
                    nc.scalar.mul(s_sb, s_ps, 1.0 / J)
                else:
                    nc.scalar.copy(s_sb, s_ps)

                # ---------- AllReduce over cores ----------
                nc.sync.dma_start(out=cc_in[it].ap(), in_=s_sb)
                nc.gpsimd.collective_compute(
                    "AllReduce", ALU.add, replica_groups=rg,
                    ins=[cc_in[it].ap()], outs=[cc_out[it].ap()],
                )
                s2 = small.tile([BS, J, O], F32)
                nc.sync.dma_start(out=s2, in_=cc_out[it].ap())

                # ---------- squash ----------
                ss = small.tile([BS, J, O], F32, tag="s_sb")
                nc.vector.tensor_tensor(out=ss, in0=s2, in1=s2, op=ALU.mult)
                sq = small.tile([BS, J], F32)
                nc.vector.tensor_reduce(out=sq, in_=ss, axis=AX.X, op=ALU.add)
                rt = small.tile([BS, J], F32)
                nc.scalar.activation(rt, sq, ACTF.Sqrt)       # sqrt(sq)
                op1 = small.tile([BS, J], F32)
                nc.scalar.add(op1, sq, 1.0)                   # 1 + sq
                den = small.tile([BS, J], F32)
                nc.vector.tensor_tensor(out=den, in0=rt, in1=op1, op=ALU.mult)
                rden = small.tile([BS, J], F32)
                nc.vector.reciprocal(rden, den)
                fac = small.tile([BS, J], F32)
                nc.vector.tensor_tensor(out=fac, in0=sq, in1=rden, op=ALU.mult)
                v_sb = small.tile([BS, J, O], F32)
                nc.vector.tensor_tensor(
                    out=v_sb, in0=s2,
                    in1=fac.unsqueeze(2).broadcast_to([BS, J, O]), op=ALU.mult)

                if it == NUM_IT - 1:
                    break

                # ---------- pass 2: b_ij += (1/BS) sum_{i,o} Wt . (uT v) ----
                v_r = small.tile([BS, JO], F32R)
                nc.vector.tensor_copy(v_r, v_sb.rearrange("p j o -> p (j o)"))
                for c in range(NCHUNK):
                    g_ps = gpsum.tile([128, JO], F32)
                    for h in range(2):
                        nc.tensor.matmul(
                            out=g_ps[:, h * 512:(h + 1) * 512],
                            lhsT=un_sb[:, c * 128:(c + 1) * 128],
                            rhs=v_r[:, h * 512:(h + 1) * 512],
                            start=True, stop=True,
                        )
                    w_c = w_sb[:, c, :].bitcast(F32)
                    if MULT_ON_GPSIMD(c):
                        g_sb = gsbp.tile([128, JO], F32)
                        nc.scalar.copy(g_sb, g_ps)
                        g_src, eng = g_sb, nc.gpsimd
                    else:
                        g_src, eng = g_ps, nc.vector
                    tmp = tmpp.tile([128, JO], F32)
                    eng.tensor_tensor(out=tmp, in0=w_c, in1=g_src, op=ALU.mult)
                    tmp3 = tmp.rearrange("p (j o) -> p j o", o=O)
                    if O_REDUCE == "mm_dup":
                        # selector matmul; o-sum via duplicated psum out AP
                        for h in range(2):
                            nc.tensor.matmul(
                                out=b_acc[:, c, h * 16:(h + 1) * 16]
                                    .unsqueeze(2).broadcast_to([128, 16, O]),
                                lhsT=sel_sb,
                                rhs=tmp3[:, h * 16:(h + 1) * 16, :],
                                start=(it == 0 and h == 0 and c % 16 == 0),
                                stop=(it == NUM_IT - 2 and h == 1
                                      and c % 16 == 15),
                                skip_group_check=True,
                            )
                    else:
                        part = tmpp.tile([128, J], F32, tag="part")
                        nc.vector.tensor_reduce(out=part, in_=tmp3, axis=AX.X,
                                                op=ALU.add)
                        nc.tensor.matmul(
                            out=b_acc[:, c, :],
                            lhsT=sel_sb,
                            rhs=part,
                            start=(it == 0 and c % 16 == 0),
                            stop=(it == NUM_IT - 2 and c % 16 == 15),
                            skip_group_check=True,
                        )

                # ---------- softmax over j (replicated rows) ----------
                nc.scalar.activation(e_rep, b_acc, ACTF.Exp)
                esum = small.tile([128, NCHUNK], F32)
                nc.vector.tensor_reduce(out=esum, in_=e_rep, axis=AX.X,
                                        op=ALU.add)
                erec = small.tile([128, NCHUNK], F32)
                nc.vector.reciprocal(erec, esum)
                nc.vector.tensor_tensor(
                    out=c_rep, in0=e_rep,
                    in1=erec.unsqueeze(2).broadcast_to([128, NCHUNK, J]),
                    op=ALU.mult)

            nc.sync.dma_start(out=vout.ap(),
                              in_=v_sb.rearrange("p j o -> p (j o)"))
    nc.finalize()
    return nc


_NC_CACHE = {}
TRACE = False            # test harness sets True for NTFF profiling
TRACE_CORES = None


def _get_nc():
    if "nc" not in _NC_CACHE:
        _NC_CACHE["nc"] = _build_nc()
    return _NC_CACHE["nc"]


def _make_sel():
    sel = np.zeros((128, 128), np.float32)
    for p in range(128):
        m0 = (p // 16) * 16
        sel[p, m0:m0 + 16] = 1.0 / BS
    return sel


def kernel(**inputs):
    in_caps = np.ascontiguousarray(inputs["in_caps"], dtype=np.float32)
    W = np.ascontiguousarray(inputs["W"], dtype=np.float32)
    assert in_caps.shape == (BS, R, I) and W.shape == (R, J, O, I)

    Wt = np.ascontiguousarray(
        W.transpose(0, 3, 1, 2).reshape(R * I, J * O))       # [(r,i), (j,o)]
    uT = np.ascontiguousarray(
        in_caps.transpose(1, 2, 0).reshape(R * I, BS))       # [(r,i), b]
    un = np.ascontiguousarray(in_caps.reshape(BS, R * I))    # [b, (r,i)]
    sel = _make_sel()

    in_maps = []
    for k in range(N_CORES):
        rows = slice(k * K_LOC, (k + 1) * K_LOC)
        in_maps.append({
            "wt": np.ascontiguousarray(Wt[rows]),
            "ut": np.ascontiguousarray(uT[rows]),
            "un": np.ascontiguousarray(un[:, rows]),
            "sel": sel,
        })

    nc = _get_nc()
    res = run_bass_kernel_spmd(nc, in_maps, core_ids=list(range(N_CORES)),
                               trace=TRACE, trace_cores=TRACE_CORES)
    _NC_CACHE["last_result"] = res
    v = np.asarray(res.results[0]["vout"], dtype=np.float32)
    return v.reshape(BS, J, O, 1)


if __name__ == "__main__":
    rng = np.random.default_rng(0)
    ins = {
        "in_caps": rng.standard_normal((BS, R, I), dtype=np.float32),
        "W": rng.standard_normal((R, J, O, I), dtype=np.float32),
    }
    out = kernel(**ins)
    print(out.shape, out.dtype, np.abs(out).mean())
